# revision 24
# baseline (speedup 1.0000x reference)
"""Fused FluxTransformerBlock on 8 Trainium2 NeuronCores.

Sharding: img tokens are split 8 ways (256 tokens/core); every core runs the
whole block for its token slice. The only cross-token dependencies are the
K/V of self-attention (img) and cross-attention (txt): each core computes
K/V for its own token shard and the shards are exchanged with AllGather.

Device-side algebra (validated against the fp32 reference by a numpy mirror):
  - LayerNorm scale/bias are folded into the projection weights on the host
    (weights-only transform); the device computes z = (x - mu) * rsqrt(var+eps)
    per token and all projections read z.  rsqrt runs as exp(-0.5*ln(x)) so the
    ScalarE table set never leaves natural_log_exp (no table reloads).
  - q is pre-scaled by DH**-0.5 (folded into Wq / cq on host).
  - RoPE(rotate-half) is applied in transposed [feature, token] layout with
    sign-folded, 2-head-stacked transposed tables.
  - Softmax runs without max subtraction (scores are O(1) by construction);
    the normalizer lambda = sum(exp(s)) comes from a ones-column appended to V,
    so P@[V|1] yields both the unnormalized output and lambda in one pass.
  - v-projection constants and o-proj biases are folded into a single
    residual bias vector added once at the end (host precomputed).

Everything runs in bf16 on the TensorEngine with fp32 PSUM accumulation;
the residual stream stays fp32.
"""
import contextlib
import ctypes
import os
import sys
import types

import ml_dtypes
import numpy as np

F32 = np.float32
BF16 = ml_dtypes.bfloat16

CFG_FULL = dict(TI=2048, TT=512, D=1024, H=16, FFN=4096, NC=8)
EPS = 1e-5
DH = 64


# --------------------------------------------------------------------------
# optional NTFF profiling under axon (enabled with KERNEL_TRACE=1)
# --------------------------------------------------------------------------
def _install_ntff_shim(so_path="/opt/axon/libaxon_pjrt.so"):
    if "antenv.axon_hooks" in sys.modules:
        return
    try:
        lib = ctypes.CDLL(so_path)
        if not hasattr(lib, "axon_start_nrt_profile"):
            return
    except OSError:
        return
    lib.axon_start_nrt_profile.argtypes = [ctypes.POINTER(ctypes.c_int64), ctypes.c_size_t]
    lib.axon_start_nrt_profile.restype = ctypes.c_int64
    lib.axon_stop_nrt_profile.argtypes = [ctypes.c_char_p]
    lib.axon_stop_nrt_profile.restype = ctypes.c_int64

    @contextlib.contextmanager
    def _hook(output_dir, device_ids):
        import jax

        jax.devices()
        if device_ids:
            ids = (ctypes.c_int64 * len(device_ids))(*device_ids)
            rc = lib.axon_start_nrt_profile(ids, len(device_ids))
        else:
            rc = lib.axon_start_nrt_profile(None, 0)
        if rc != 0:
            raise RuntimeError(f"axon_start_nrt_profile rc={rc}")
        try:
            yield
        finally:
            n = lib.axon_stop_nrt_profile(str(output_dir).encode())
            print(f"ntff profile: {n} file(s) -> {output_dir}", file=sys.stderr)

    mod = types.ModuleType("antenv.axon_hooks")
    mod.get_axon_ntff_profile_hook = lambda: _hook
    mod.set_axon_ntff_profile_hook = lambda h: None
    sys.modules["antenv.axon_hooks"] = mod


# --------------------------------------------------------------------------
# host-side weight folding (weights/constants only — no activation compute)
# --------------------------------------------------------------------------
def host_prep(params, sin_img, cos_img, sin_txt, cos_txt, cfg=CFG_FULL):
    D = cfg["D"]
    p = {k: np.asarray(v, F32) for k, v in params.items()}
    s = DH ** -0.5
    o = {}
    o["Wq_sa"] = (p["ln1_w"][:, None] * p["sa_q_w"]) * s
    o["cq_sa"] = (p["ln1_b"] @ p["sa_q_w"] + p["sa_q_b"]) * s
    o["Wk_sa"] = p["ln1_w"][:, None] * p["sa_kv_w"][:, :D]
    o["ck_sa"] = p["ln1_b"] @ p["sa_kv_w"][:, :D] + p["sa_kv_b"][:D]
    o["Wv_sa"] = p["ln1_w"][:, None] * p["sa_kv_w"][:, D:]
    cv_sa = p["ln1_b"] @ p["sa_kv_w"][:, D:] + p["sa_kv_b"][D:]
    o["Wo_sa"] = p["sa_o_w"]
    o["Wq_ca"] = (p["ln2_w"][:, None] * p["ca_q_w"]) * s
    o["cq_ca"] = (p["ln2_b"] @ p["ca_q_w"] + p["ca_q_b"]) * s
    o["Wk_ca"] = p["ln2_w"][:, None] * p["ca_kv_w"][:, :D]
    o["ck_ca"] = p["ln2_b"] @ p["ca_kv_w"][:, :D] + p["ca_kv_b"][:D]
    o["Wv_ca"] = p["ln2_w"][:, None] * p["ca_kv_w"][:, D:]
    cv_ca = p["ln2_b"] @ p["ca_kv_w"][:, D:] + p["ca_kv_b"][D:]
    o["Wo_ca"] = p["ca_o_w"]
    o["W1"] = p["ln3_w"][:, None] * p["ffn1_w"]
    o["ch"] = p["ln3_b"] @ p["ffn1_w"] + p["ffn1_b"]
    o["W2"] = p["ffn2_w"]
    o["c_bias"] = (
        p["sa_o_b"] + p["ca_o_b"] + p["ffn2_b"]
        + cv_sa @ p["sa_o_w"] + cv_ca @ p["ca_o_w"]
    )

    def rope_tabs(sin, cos):
        sinT = np.asarray(sin, F32).T
        cosT = np.asarray(cos, F32).T
        ssg = np.concatenate([-sinT[0:32], sinT[32:64]], 0)
        return np.concatenate([ssg, ssg], 0), np.concatenate([cosT, cosT], 0)

    o["sinT_img"], o["cosT_img"] = rope_tabs(sin_img, cos_img)
    o["sinT_txt"], o["cosT_txt"] = rope_tabs(sin_txt, cos_txt)
    return o


def lhs_pack(W):
    """[D, F] -> [F//128, 128, D//128, 128]: out[ft,p,j,f] = W[j*128+p, ft*128+f]."""
    D, F = W.shape
    return np.ascontiguousarray(
        W.reshape(D // 128, 128, F // 128, 128).transpose(2, 1, 0, 3))


def rhs_pack(W, fc=512):
    """[D, F] -> [F//fc, 128, D//128, fc]: out[c,p,j,f] = W[j*128+p, c*fc+f]."""
    D, F = W.shape
    return np.ascontiguousarray(
        W.reshape(D // 128, 128, F // fc, fc).transpose(2, 1, 0, 3))


# --------------------------------------------------------------------------
# device program
# --------------------------------------------------------------------------
def build(cfg=CFG_FULL):
    import concourse.bass as bass
    import concourse.tile as tile
    from concourse import bacc, mybir
    from concourse.masks import make_identity

    f32 = mybir.dt.float32
    bf16 = mybir.dt.bfloat16
    AF = mybir.ActivationFunctionType
    ALU = mybir.AluOpType

    D, H, FFN, TI, TT, NC = (cfg[k] for k in ("D", "H", "FFN", "TI", "TT", "NC"))
    TPC, TTPC = TI // NC, TT // NC
    JD, JF = D // 128, FFN // 128
    NT = TPC // 128          # img token tiles per core (2)
    HP = H // 2              # head pairs
    E = DH + 1               # v_ext row width per head
    NFC = D // 512           # rhs 512-chunks of D

    nc = bacc.Bacc(None, target_bir_lowering=False, num_devices=NC)

    # ---- I/O ----
    ximg = nc.declare_dram_parameter("ximg", [TPC, D], f32, isOutput=False)
    xtxt = nc.declare_dram_parameter("xtxt", [TTPC, D], f32, isOutput=False)
    cosq = nc.declare_dram_parameter("cosq", [128, TPC], bf16, isOutput=False)
    sinq = nc.declare_dram_parameter("sinq", [128, TPC], bf16, isOutput=False)
    coskt = nc.declare_dram_parameter("coskt", [128, TTPC], bf16, isOutput=False)
    sinkt = nc.declare_dram_parameter("sinkt", [128, TTPC], bf16, isOutput=False)
    W = {}
    for n in ("Wq_sa", "Wk_sa", "Wq_ca", "Wk_ca"):
        W[n] = nc.declare_dram_parameter(n, [JD, 128, JD, 128], bf16, isOutput=False)
    W["W1"] = nc.declare_dram_parameter("W1", [JF, 128, JD, 128], bf16, isOutput=False)
    for n in ("Wv_sa", "Wo_sa", "Wv_ca", "Wo_ca"):
        W[n] = nc.declare_dram_parameter(n, [NFC, 128, JD, 512], bf16, isOutput=False)
    W["W2"] = nc.declare_dram_parameter("W2", [FFN, D], bf16, isOutput=False)
    cq_sa = nc.declare_dram_parameter("cq_sa", [128, JD], f32, isOutput=False)
    ck_sa = nc.declare_dram_parameter("ck_sa", [128, JD], f32, isOutput=False)
    cq_ca = nc.declare_dram_parameter("cq_ca", [128, JD], f32, isOutput=False)
    ck_ca = nc.declare_dram_parameter("ck_ca", [128, JD], f32, isOutput=False)
    ch = nc.declare_dram_parameter("ch", [128, JF], f32, isOutput=False)
    c_bias = nc.declare_dram_parameter("c_bias", [1, D], f32, isOutput=False)
    img_out = nc.declare_dram_parameter("img_out", [TPC, D], f32, isOutput=True)

    # ---- internal DRAM (collective bounce) ----
    HD2 = D // 2
    HE2 = (H // 2) * (DH + 1)
    agk_in = [nc.dram_tensor(f"agk_in{h}", [HD2, TPC], bf16) for h in range(2)]
    agk_out = [nc.dram_tensor(f"agk_out{h}", [NC * HD2, TPC], bf16, addr_space="Shared")
               for h in range(2)]
    agv_in = [nc.dram_tensor(f"agv_in{h}", [TPC, HE2], bf16) for h in range(2)]
    agv_out = [nc.dram_tensor(f"agv_out{h}", [NC * TPC, HE2], bf16, addr_space="Shared")
               for h in range(2)]
    agkt_in = nc.dram_tensor("agkt_in", [D, TTPC], bf16)
    agkt_out = nc.dram_tensor("agkt_out", [NC * D, TTPC], bf16, addr_space="Shared")
    agvt_in = nc.dram_tensor("agvt_in", [TTPC, H * E], bf16)
    agvt_out = nc.dram_tensor("agvt_out", [NC * TTPC, H * E], bf16, addr_space="Shared")
    rg = [list(range(NC))]

    with tile.TileContext(nc) as tc, contextlib.ExitStack() as ctx:
        scope = nc.named_scope
        sing = ctx.enter_context(tc.tile_pool(name="sing", bufs=1))
        big = ctx.enter_context(tc.tile_pool(name="big", bufs=1))
        dbl = ctx.enter_context(tc.tile_pool(name="dbl", bufs=2))
        wp = ctx.enter_context(tc.tile_pool(name="wp", bufs=2))
        work = ctx.enter_context(tc.tile_pool(name="work", bufs=4))
        psA = ctx.enter_context(tc.tile_pool(name="psA", bufs=2, space="PSUM"))
        psPV = ctx.enter_context(tc.tile_pool(name="psPV", bufs=2, space="PSUM"))

        # ---- constants ----
        ident = sing.tile([128, 128], bf16, tag="ident")
        make_identity(nc, ident)
        eps_t = sing.tile([128, 1], f32, tag="eps")
        nc.vector.memset(eps_t, EPS)
        cosq_t = sing.tile([128, TPC], bf16, tag="cosq")
        sinq_t = sing.tile([128, TPC], bf16, tag="sinq")
        nc.sync.dma_start(out=cosq_t, in_=cosq[:, :])
        nc.sync.dma_start(out=sinq_t, in_=sinq[:, :])
        coskt_t = sing.tile([128, TTPC], bf16, tag="coskt")
        sinkt_t = sing.tile([128, TTPC], bf16, tag="sinkt")
        nc.sync.dma_start(out=coskt_t, in_=coskt[:, :])
        nc.sync.dma_start(out=sinkt_t, in_=sinkt[:, :])
        cq_sa_t = sing.tile([128, JD], f32, tag="cq_sa")
        ck_sa_t = sing.tile([128, JD], f32, tag="ck_sa")
        cq_ca_t = sing.tile([128, JD], f32, tag="cq_ca")
        ck_ca_t = sing.tile([128, JD], f32, tag="ck_ca")
        ch_t = sing.tile([128, JF], f32, tag="ch")
        for t, h in ((cq_sa_t, cq_sa), (ck_sa_t, ck_sa), (cq_ca_t, cq_ca), (ck_ca_t, ck_ca), (ch_t, ch)):
            nc.sync.dma_start(out=t, in_=h[:, :])
        cbias_t = sing.tile([128, D], f32, tag="cbias")
        nc.sync.dma_start(out=cbias_t, in_=c_bias[0:1, :].to_broadcast((128, D)))

        # ---- residual stream ----
        ximg_sb = big.tile([128, NT, D], f32, tag="ximg")
        nc.sync.dma_start(out=ximg_sb, in_=ximg[:, :].rearrange("(i p) d -> p i d", p=128))
        xtxt_sb = big.tile([TTPC, D], f32, tag="xtxt")
        nc.sync.dma_start(out=xtxt_sb, in_=xtxt[:, :])

        # ---- helpers ----
        def standardize(x_ap, P, z_out):
            """z_out[:P,:] = (x - mean) * rsqrt(var + eps), bf16.

            rsqrt as exp(-0.5*ln(var+eps)) keeps ScalarE on one table set."""
            nsub = D // 512
            st = work.tile([128, nsub, 6], f32, tag="bnst")
            for s_ in range(nsub):
                nc.vector.bn_stats(out=st[0:P, s_, :], in_=x_ap[:, s_ * 512:(s_ + 1) * 512])
            mv = work.tile([128, 2], f32, tag="bnmv")
            nc.vector.bn_aggr(out=mv[0:P], in_=st[0:P])
            lg = work.tile([128, 1], f32, tag="bnlg")
            nc.scalar.activation(out=lg[0:P], in_=mv[0:P, 1:2], func=AF.Sqrt, bias=eps_t[0:P], scale=1.0)
            a = work.tile([128, 1], f32, tag="bna")
            nc.vector.reciprocal(out=a[0:P], in_=lg[0:P])
            nmua = work.tile([128, 1], f32, tag="bnnm")
            nc.vector.tensor_scalar(out=nmua[0:P], in0=mv[0:P, 0:1], scalar1=a[0:P],
                                    scalar2=-1.0, op0=ALU.mult, op1=ALU.mult)
            nc.vector.tensor_scalar(out=z_out[0:P], in0=x_ap, scalar1=a[0:P],
                                    scalar2=nmua[0:P], op0=ALU.mult, op1=ALU.add)

        def make_zT(x_sb, ntile, P, T, ztag):
            """LN-standardize x_sb [128, ntile, D] then transpose -> [128, JD, T] bf16."""
            zT = dbl.tile([128, JD, T], bf16, tag=ztag)
            for i in range(ntile):
                z = dbl.tile([128, D], bf16, tag="znat")
                x_ap = x_sb[:, i, :] if len(x_sb.shape) == 3 else x_sb[:, :]
                standardize(x_ap, P, z)
                for j in range(JD):
                    pt = psPV.tile([128, 128], bf16, tag="pvtr")
                    nc.tensor.transpose(pt[0:128, 0:P], z[0:P, j * 128:(j + 1) * 128], ident[0:P, 0:P])
                    nc.vector.tensor_copy(out=zT[:, j, i * 128:i * 128 + P], in_=pt[0:128, 0:P])
            return zT

        def rope_apply(out_ap, x_ap, sin_t, cos_t, T):
            rot = work.tile([128, T], bf16, tag="roperot", bufs=2)
            for h0 in (0, 64):
                nc.vector.tensor_copy(out=rot[h0:h0 + 32, :], in_=x_ap[h0 + 32:h0 + 64, :])
                nc.vector.tensor_copy(out=rot[h0 + 32:h0 + 64, :], in_=x_ap[h0:h0 + 32, :])
            nc.vector.tensor_mul(rot, rot, sin_t)
            nc.vector.tensor_mul(out_ap, x_ap, cos_t)
            nc.vector.tensor_add(out_ap, out_ap, rot)

        def proj_T(zT, w_handle, c_t, sin_t, cos_t, T, out, wtag, fts=None):
            """out[:, ft, :T] = rope(W.T @ z + c) for each 128-feature tile."""
            for ft in (range(JD) if fts is None else fts):
                wl = wp.tile([128, JD, 128], bf16, tag=wtag, name=f"wl_{wtag}_{ft}")
                nc.sync.dma_start(out=wl, in_=w_handle[ft])
                ps = psA.tile([128, T], f32, tag="mm", name=f"pp_{wtag}_{ft}")
                for j in range(JD):
                    nc.tensor.matmul(ps, wl[:, j, :], zT[:, j, :], start=(j == 0), stop=(j == JD - 1))
                raw = work.tile([128, T], bf16, tag="praw", bufs=2, name=f"raw_{wtag}_{ft}")
                nc.scalar.activation(out=raw, in_=ps, func=AF.Identity, bias=c_t[:, ft:ft + 1])
                rope_apply(out[:, ft, :], raw, sin_t, cos_t, T)

        def vproj_ext(zT, w_handle, ntile, P, vtag, wtag, half_cb=None, fcs=None, tiles=None):
            """v_ext tiles [128, H*E] bf16 (natural layout + ones col per head).

            fc-outer so the first head-half of every token tile completes first;
            half_cb(fc, tiles) fires when a half is fully written (for split AGs)."""
            if tiles is None:
                tiles = []
                for i in range(ntile):
                    ve = dbl.tile([128, H * E], bf16, tag=vtag, name=f"ve_{vtag}_{i}_{wtag}")
                    nc.vector.memset(ve.rearrange("p (h e) -> p h e", e=E)[:, :, DH:E], 1.0)
                    tiles.append(ve)
            for fc in (range(NFC) if fcs is None else fcs):
                wv = wp.tile([128, JD, 512], bf16, tag=wtag, name=f"wv_{wtag}_{fc}_{id(zT)}")
                nc.sync.dma_start(out=wv, in_=w_handle[fc])
                for i in range(ntile):
                    ve = tiles[i]
                    ps = psA.tile([128, 512], f32, tag="mm", name=f"vp_{vtag}_{i}_{fc}")
                    for j in range(JD):
                        nc.tensor.matmul(ps[0:P, :], zT[:, j, i * 128:i * 128 + P], wv[:, j, :],
                                         start=(j == 0), stop=(j == JD - 1))
                    for hh in range(512 // DH):
                        h = fc * (512 // DH) + hh
                        nc.vector.tensor_copy(out=ve[0:P, h * E:h * E + DH],
                                              in_=ps[0:P, hh * DH:(hh + 1) * DH])
                if half_cb is not None:
                    half_cb(fc, tiles)
            return tiles

        # ================== SA K/V (own tokens) + split AllGathers ==================
        with scope("ln1"):
            z1T = make_zT(ximg_sb, NT, 128, TPC, "zT1")
        kT_own = dbl.tile([128, JD, TPC], bf16, tag="kTo")
        v_tiles = []
        for i in range(NT):
            ve = dbl.tile([128, H * E], bf16, tag="vext", name=f"ve_sa_{i}")
            nc.vector.memset(ve.rearrange("p (h e) -> p h e", e=E)[:, :, DH:E], 1.0)
            v_tiles.append(ve)

        def k_half(hh):
            nc.sync.dma_start(
                out=agk_in[hh][:, :].rearrange("(j p) t -> p j t", p=128),
                in_=kT_own[:, hh * (JD // 2):(hh + 1) * (JD // 2), :])
            nc.gpsimd.collective_compute("AllGather", ALU.bypass, replica_groups=rg,
                                         ins=[agk_in[hh][:, :].opt()],
                                         outs=[agk_out[hh][:, :].opt()])

        def v_half_done(fc, tiles):
            for i, ve in enumerate(tiles):
                nc.sync.dma_start(out=agv_in[fc][i * 128:(i + 1) * 128, :],
                                  in_=ve[:, fc * HE2:(fc + 1) * HE2])
            nc.gpsimd.collective_compute("AllGather", ALU.bypass, replica_groups=rg,
                                         ins=[agv_in[fc][:, :].opt()],
                                         outs=[agv_out[fc][:, :].opt()])

        with scope("kv_sa"):
            proj_T(z1T, W["Wk_sa"], ck_sa_t, sinq_t, cosq_t, TPC, kT_own, "wk", fts=range(0, JD // 2))
        k_half(0)
        with scope("v_sa"):
            vproj_ext(z1T, W["Wv_sa"], NT, 128, "vext", "wv", half_cb=v_half_done,
                      fcs=[0], tiles=v_tiles)
        with scope("kv_sa"):
            proj_T(z1T, W["Wk_sa"], ck_sa_t, sinq_t, cosq_t, TPC, kT_own, "wk", fts=range(JD // 2, JD))
        k_half(1)
        with scope("v_sa"):
            vproj_ext(z1T, W["Wv_sa"], NT, 128, "vext", "wv", half_cb=v_half_done,
                      fcs=[1], tiles=v_tiles)
        # ================== SA q (overlaps AGs) ==================
        qT = dbl.tile([128, JD, TPC], bf16, tag="qT")
        with scope("q_sa"):
            proj_T(z1T, W["Wq_sa"], cq_sa_t, sinq_t, cosq_t, TPC, qT, "wk")

        # ================== CA txt K/V (fills the AG wait window) ==================
        with scope("txtkv"):
            z2tT = make_zT(xtxt_sb, 1, TTPC, TTPC, "zTt")
            ktT_own = dbl.tile([128, JD, TTPC], bf16, tag="kTo")
            proj_T(z2tT, W["Wk_ca"], ck_ca_t, sinkt_t, coskt_t, TTPC, ktT_own, "wk")
        nc.sync.dma_start(out=agkt_in[:, :].rearrange("(j p) t -> p j t", p=128), in_=ktT_own)
        nc.gpsimd.collective_compute("AllGather", ALU.bypass, replica_groups=rg,
                                     ins=[agkt_in[:, :].opt()], outs=[agkt_out[:, :].opt()])
        with scope("txtkv"):
            vt_tiles = vproj_ext(z2tT, W["Wv_ca"], 1, TTPC, "vext", "wv")
        nc.sync.dma_start(out=agvt_in[:, :], in_=vt_tiles[0][0:TTPC, :])
        nc.gpsimd.collective_compute("AllGather", ALU.bypass, replica_groups=rg,
                                     ins=[agvt_in[:, :].opt()], outs=[agvt_out[:, :].opt()])

        # ================== gathered K/V (per half) ==================
        KH = []
        for hh in range(2):
            ka = big.tile([128, JD // 2, NC, TPC], bf16, tag=f"KALL{hh}", name=f"KALL{hh}")
            for cc in range(NC):
                nc.sync.dma_start(out=ka[:, :, cc, :],
                                  in_=agk_out[hh][cc * HD2:(cc + 1) * HD2, :]
                                  .rearrange("(j p) t -> p j t", p=128))
            KH.append(ka)
        VH = []
        for hh in range(2):
            va = big.tile([128, NC * NT, HE2], bf16, tag=f"VALL{hh}", name=f"VALL{hh}")
            nc.sync.dma_start(out=va, in_=agv_out[hh][:, :].rearrange("(u p) e -> p u e", p=128))
            VH.append(va)
        KALL, VALL = KH, VH

        def attention(qT_t, KA, VA, n_tok_blk, attnT):
            split = isinstance(KA, list)
            for g in range(HP):
                h0, h1 = 2 * g, 2 * g + 1
                if split:
                    KAg = KA[g // (HP // 2)]
                    VAg = VA[g // (HP // 2)]
                    gg, hh0, hh1 = g % (HP // 2), h0 % (H // 2), h1 % (H // 2)
                else:
                    KAg, VAg, gg, hh0, hh1 = KA, VA, g, h0, h1
                pv0 = psPV.tile([DH + 1, TPC], f32, tag="pvtr", name=f"pv0_{g}")
                pv1 = psPV.tile([DH + 1, TPC], f32, tag="pvtr", name=f"pv1_{g}")
                if n_tok_blk >= 128:
                    sub = n_tok_blk // 128
                    ntile = NC * sub
                    pTs = []
                    for b in range(ntile):
                        cc, jj = b // sub, b % sub
                        ps = psA.tile([128, 2, 512], f32, tag="sc2", name=f"sc_{g}_{b}")
                        ksl = KAg[:, gg, cc, jj * 128:(jj + 1) * 128]
                        nc.tensor.matmul(ps[:, 0, 0:TPC], ksl[0:64, :], qT_t[0:64, g, :], start=True, stop=True)
                        nc.tensor.matmul(ps[:, 1, 0:TPC], ksl[64:128, :], qT_t[64:128, g, :], start=True, stop=True)
                        pT = work.tile([128, 2 * TPC], bf16, tag="pT", bufs=NC * NT + 1,
                                       name=f"pt_{g}_{b}")
                        nc.scalar.activation(out=pT, in_=ps[:, :, 0:TPC], func=AF.Exp)
                        pTs.append(pT)
                    for b in range(ntile):
                        cc, jj = b // sub, b % sub
                        u = (cc * n_tok_blk) // 128 + jj
                        nc.tensor.matmul(pv0, VAg[:, u, hh0 * E:(hh0 + 1) * E], pTs[b][:, 0:TPC],
                                         start=(b == 0), stop=(b == ntile - 1))
                        nc.tensor.matmul(pv1, VAg[:, u, hh1 * E:(hh1 + 1) * E], pTs[b][:, TPC:2 * TPC],
                                         start=(b == 0), stop=(b == ntile - 1))
                else:
                    assert n_tok_blk == 64
                    pTs = []
                    for b2 in range(NC // 2):
                        ps = psA.tile([128, 2, 512], f32, tag="sc2", name=f"scc_{g}_{b2}")
                        for s_ in range(2):
                            cc = 2 * b2 + s_
                            ksl = KAg[:, gg, cc, :]
                            nc.tensor.matmul(ps[s_ * 64:(s_ + 1) * 64, 0, 0:TPC], ksl[0:64, :],
                                             qT_t[0:64, g, :], start=True, stop=True,
                                             tile_position=(0, s_ * 64))
                            nc.tensor.matmul(ps[s_ * 64:(s_ + 1) * 64, 1, 0:TPC], ksl[64:128, :],
                                             qT_t[64:128, g, :], start=True, stop=True,
                                             tile_position=(64, s_ * 64))
                        pT = work.tile([128, 2 * TPC], bf16, tag="pT", bufs=NC * NT + 1,
                                       name=f"ptc_{g}_{b2}")
                        nc.scalar.activation(out=pT, in_=ps[:, :, 0:TPC], func=AF.Exp)
                        pTs.append(pT)
                    for b2 in range(NC // 2):
                        nc.tensor.matmul(pv0, VAg[:, b2, hh0 * E:(hh0 + 1) * E], pTs[b2][:, 0:TPC],
                                         start=(b2 == 0), stop=(b2 == NC // 2 - 1))
                        nc.tensor.matmul(pv1, VAg[:, b2, hh1 * E:(hh1 + 1) * E], pTs[b2][:, TPC:2 * TPC],
                                         start=(b2 == 0), stop=(b2 == NC // 2 - 1))
                for h, pv in ((h0, pv0), (h1, pv1)):
                    rec = work.tile([1, TPC], f32, tag="rec", name=f"rec_{g}_{h}")
                    nc.vector.reciprocal(out=rec, in_=pv[DH:DH + 1, :])
                    recb = work.tile([64, TPC], f32, tag="recb", bufs=2, name=f"recb_{g}_{h}")
                    nc.gpsimd.partition_broadcast(recb, rec[0:1, :])
                    nc.vector.tensor_mul(attnT[(h % 2) * 64:(h % 2) * 64 + 64, h // 2, :],
                                         pv[0:DH, :], recb)

        def oproj_residual(attnT, w_handle):
            for fc in range(NFC):
                wo = wp.tile([128, JD, 512], bf16, tag="wv", name=f"wo_{fc}")
                nc.sync.dma_start(out=wo, in_=w_handle[fc])
                for i in range(NT):
                    ps = psA.tile([128, 512], f32, tag="mm", name=f"op_{fc}_{i}")
                    for j in range(JD):
                        nc.tensor.matmul(ps, attnT[:, j, i * 128:(i + 1) * 128], wo[:, j, :],
                                         start=(j == 0), stop=(j == JD - 1))
                    nc.vector.tensor_add(ximg_sb[:, i, fc * 512:(fc + 1) * 512],
                                         ximg_sb[:, i, fc * 512:(fc + 1) * 512], ps)

        # ================== SA attention + o-proj ==================
        attnT = dbl.tile([128, JD, TPC], bf16, tag="attnT")
        with scope("sa_attn"):
            attention(qT, KALL, VALL, TPC, attnT)
        with scope("sa_oproj"):
            oproj_residual(attnT, W["Wo_sa"])

        # ================== CA ==================
        with scope("ln2"):
            z2T = make_zT(ximg_sb, NT, 128, TPC, "zT1")
        qTc = dbl.tile([128, JD, TPC], bf16, tag="qT")
        with scope("q_ca"):
            proj_T(z2T, W["Wq_ca"], cq_ca_t, sinq_t, cosq_t, TPC, qTc, "wk")
        KTALL = big.tile([128, JD, NC, TTPC], bf16, tag="KTALL")
        for cc in range(NC):
            nc.sync.dma_start(out=KTALL[:, :, cc, :], in_=agkt_out[cc * D:(cc + 1) * D, :]
                              .rearrange("(j p) t -> p j t", p=128))
        VTALL = big.tile([128, (NC * TTPC) // 128, H * E], bf16, tag="VTALL")
        nc.sync.dma_start(out=VTALL, in_=agvt_out[:, :].rearrange("(u p) e -> p u e", p=128))
        attnTc = dbl.tile([128, JD, TPC], bf16, tag="attnT")
        with scope("ca_attn"):
            attention(qTc, KTALL, VTALL, TTPC, attnTc)
        with scope("ca_oproj"):
            oproj_residual(attnTc, W["Wo_ca"])

        # ================== FFN ==================
        with scope("ln3"):
            z3T = make_zT(ximg_sb, NT, 128, TPC, "zT1")
        hT = big.tile([128, JF, TPC], bf16, tag="hT")
        with scope("ffn1"):
            for ft in range(JF):
                wl = wp.tile([128, JD, 128], bf16, tag="wk", name=f"w1_{ft}")
                nc.sync.dma_start(out=wl, in_=W["W1"][ft])
                ps = psA.tile([128, TPC], f32, tag="mm", name=f"f1_{ft}")
                for j in range(JD):
                    nc.tensor.matmul(ps, wl[:, j, :], z3T[:, j, :], start=(j == 0), stop=(j == JD - 1))
                nc.scalar.activation(out=hT[:, ft, :], in_=ps, func=AF.Silu,
                                     bias=ch_t[:, ft:ft + 1], scale=1.0)
        with scope("ffn2"):
            acct = [psA.tile([128, NFC, 512], f32, tag="sc2", name=f"acct_{i}")
                    for i in range(NT)]
            accs = [[acct[i][:, fc, :] for fc in range(NFC)] for i in range(NT)]
            for j in range(JF):
                w2t = wp.tile([128, D], bf16, tag="w2t", name=f"w2_{j}")
                nc.sync.dma_start(out=w2t, in_=W["W2"][j * 128:(j + 1) * 128, :])
                for i in range(NT):
                    for fc in range(NFC):
                        nc.tensor.matmul(accs[i][fc], hT[:, j, i * 128:(i + 1) * 128],
                                         w2t[:, fc * 512:(fc + 1) * 512],
                                         start=(j == 0), stop=(j == JF - 1))
            for i in range(NT):
                for fc in range(NFC):
                    nc.vector.tensor_add(ximg_sb[:, i, fc * 512:(fc + 1) * 512],
                                         ximg_sb[:, i, fc * 512:(fc + 1) * 512], accs[i][fc])
                nc.vector.tensor_add(ximg_sb[:, i, :], ximg_sb[:, i, :], cbias_t)
                nc.sync.dma_start(out=img_out[i * 128:(i + 1) * 128, :], in_=ximg_sb[:, i, :])

    nc.compile()
    return nc


# --------------------------------------------------------------------------
# per-core input maps
# --------------------------------------------------------------------------
def make_in_maps(img_seq, text_seq, w, cfg=CFG_FULL):
    D, H, FFN, TI, TT, NC = (cfg[k] for k in ("D", "H", "FFN", "TI", "TT", "NC"))
    TPC, TTPC = TI // NC, TT // NC
    img = np.asarray(img_seq, F32).reshape(TI, D)
    txt = np.asarray(text_seq, F32).reshape(TT, D)

    def pack_c(v):
        return np.ascontiguousarray(v.reshape(-1, 128).T.astype(F32))

    shared = {
        "Wq_sa": lhs_pack(w["Wq_sa"].astype(BF16)), "Wk_sa": lhs_pack(w["Wk_sa"].astype(BF16)),
        "Wq_ca": lhs_pack(w["Wq_ca"].astype(BF16)), "Wk_ca": lhs_pack(w["Wk_ca"].astype(BF16)),
        "W1": lhs_pack(w["W1"].astype(BF16)),
        "Wv_sa": rhs_pack(w["Wv_sa"].astype(BF16)), "Wo_sa": rhs_pack(w["Wo_sa"].astype(BF16)),
        "Wv_ca": rhs_pack(w["Wv_ca"].astype(BF16)), "Wo_ca": rhs_pack(w["Wo_ca"].astype(BF16)),
        "W2": np.ascontiguousarray(w["W2"].astype(BF16)),
        "cq_sa": pack_c(w["cq_sa"]), "ck_sa": pack_c(w["ck_sa"]),
        "cq_ca": pack_c(w["cq_ca"]), "ck_ca": pack_c(w["ck_ca"]),
        "ch": pack_c(w["ch"]), "c_bias": w["c_bias"].reshape(1, D).astype(F32),
    }
    shared = {k: np.ascontiguousarray(v) for k, v in shared.items()}
    maps = []
    for c in range(NC):
        m = dict(shared)
        m["ximg"] = np.ascontiguousarray(img[c * TPC:(c + 1) * TPC])
        m["xtxt"] = np.ascontiguousarray(txt[c * TTPC:(c + 1) * TTPC])
        m["cosq"] = np.ascontiguousarray(w["cosT_img"][:, c * TPC:(c + 1) * TPC].astype(BF16))
        m["sinq"] = np.ascontiguousarray(w["sinT_img"][:, c * TPC:(c + 1) * TPC].astype(BF16))
        m["coskt"] = np.ascontiguousarray(w["cosT_txt"][:, c * TTPC:(c + 1) * TTPC].astype(BF16))
        m["sinkt"] = np.ascontiguousarray(w["sinT_txt"][:, c * TTPC:(c + 1) * TTPC].astype(BF16))
        maps.append(m)
    return maps


_CACHE = {}
last_exec_ns = None
last_result = None


def kernel(img_seq, text_seq, sin_img, cos_img, sin_txt, cos_txt, params):
    global last_exec_ns, last_result
    cfg = CFG_FULL
    trace = os.environ.get("KERNEL_TRACE", "0") == "1"
    if trace:
        _install_ntff_shim()
    from concourse.bass_utils import run_bass_kernel_spmd

    if "nc" not in _CACHE:
        _CACHE["nc"] = build(cfg)
    nc = _CACHE["nc"]
    w = host_prep(params, sin_img, cos_img, sin_txt, cos_txt, cfg)
    in_maps = make_in_maps(img_seq, text_seq, w, cfg)
    res = run_bass_kernel_spmd(nc, in_maps, core_ids=list(range(cfg["NC"])), trace=trace)
    last_result = res
    last_exec_ns = res.exec_time_ns
    TI, D, NC = cfg["TI"], cfg["D"], cfg["NC"]
    TPC = TI // NC
    img = np.concatenate([res.results[c]["img_out"] for c in range(NC)], 0)
    img = img.reshape(1, TI, D).astype(F32)
    txt = np.asarray(text_seq, F32)
    return img, txt


# revision 25
# speedup vs baseline: 1.0527x; 1.0527x over previous
"""Fused FluxTransformerBlock on 8 Trainium2 NeuronCores.

Sharding: img tokens are split 8 ways (256 tokens/core); every core runs the
whole block for its token slice. The only cross-token dependencies are the
K/V of self-attention (img) and cross-attention (txt): each core computes
K/V for its own token shard and the shards are exchanged with AllGather.

Device-side algebra (validated against the fp32 reference by a numpy mirror):
  - LayerNorm scale/bias are folded into the projection weights on the host
    (weights-only transform); the device computes z = (x - mu) * rsqrt(var+eps)
    per token and all projections read z.  rsqrt runs as exp(-0.5*ln(x)) so the
    ScalarE table set never leaves natural_log_exp (no table reloads).
  - q is pre-scaled by DH**-0.5 (folded into Wq / cq on host).
  - RoPE(rotate-half) is applied in transposed [feature, token] layout with
    sign-folded, 2-head-stacked transposed tables.
  - Softmax runs without max subtraction (scores are O(1) by construction);
    the normalizer lambda = sum(exp(s)) comes from a ones-column appended to V,
    so P@[V|1] yields both the unnormalized output and lambda in one pass.
  - v-projection constants and o-proj biases are folded into a single
    residual bias vector added once at the end (host precomputed).

Everything runs in bf16 on the TensorEngine with fp32 PSUM accumulation;
the residual stream stays fp32.
"""
import contextlib
import ctypes
import os
import sys
import types

import ml_dtypes
import numpy as np

F32 = np.float32
BF16 = ml_dtypes.bfloat16

CFG_FULL = dict(TI=2048, TT=512, D=1024, H=16, FFN=4096, NC=8)
EPS = 1e-5
DH = 64


# --------------------------------------------------------------------------
# optional NTFF profiling under axon (enabled with KERNEL_TRACE=1)
# --------------------------------------------------------------------------
def _install_ntff_shim(so_path="/opt/axon/libaxon_pjrt.so"):
    if "antenv.axon_hooks" in sys.modules:
        return
    try:
        lib = ctypes.CDLL(so_path)
        if not hasattr(lib, "axon_start_nrt_profile"):
            return
    except OSError:
        return
    lib.axon_start_nrt_profile.argtypes = [ctypes.POINTER(ctypes.c_int64), ctypes.c_size_t]
    lib.axon_start_nrt_profile.restype = ctypes.c_int64
    lib.axon_stop_nrt_profile.argtypes = [ctypes.c_char_p]
    lib.axon_stop_nrt_profile.restype = ctypes.c_int64

    @contextlib.contextmanager
    def _hook(output_dir, device_ids):
        import jax

        jax.devices()
        if device_ids:
            ids = (ctypes.c_int64 * len(device_ids))(*device_ids)
            rc = lib.axon_start_nrt_profile(ids, len(device_ids))
        else:
            rc = lib.axon_start_nrt_profile(None, 0)
        if rc != 0:
            raise RuntimeError(f"axon_start_nrt_profile rc={rc}")
        try:
            yield
        finally:
            n = lib.axon_stop_nrt_profile(str(output_dir).encode())
            print(f"ntff profile: {n} file(s) -> {output_dir}", file=sys.stderr)

    mod = types.ModuleType("antenv.axon_hooks")
    mod.get_axon_ntff_profile_hook = lambda: _hook
    mod.set_axon_ntff_profile_hook = lambda h: None
    sys.modules["antenv.axon_hooks"] = mod


# --------------------------------------------------------------------------
# host-side weight folding (weights/constants only — no activation compute)
# --------------------------------------------------------------------------
def host_prep(params, sin_img, cos_img, sin_txt, cos_txt, cfg=CFG_FULL):
    D = cfg["D"]
    p = {k: np.asarray(v, F32) for k, v in params.items()}
    s = DH ** -0.5
    o = {}
    o["Wq_sa"] = (p["ln1_w"][:, None] * p["sa_q_w"]) * s
    o["cq_sa"] = (p["ln1_b"] @ p["sa_q_w"] + p["sa_q_b"]) * s
    o["Wk_sa"] = p["ln1_w"][:, None] * p["sa_kv_w"][:, :D]
    o["ck_sa"] = p["ln1_b"] @ p["sa_kv_w"][:, :D] + p["sa_kv_b"][:D]
    o["Wv_sa"] = p["ln1_w"][:, None] * p["sa_kv_w"][:, D:]
    cv_sa = p["ln1_b"] @ p["sa_kv_w"][:, D:] + p["sa_kv_b"][D:]
    o["Wo_sa"] = p["sa_o_w"]
    o["Wq_ca"] = (p["ln2_w"][:, None] * p["ca_q_w"]) * s
    o["cq_ca"] = (p["ln2_b"] @ p["ca_q_w"] + p["ca_q_b"]) * s
    o["Wk_ca"] = p["ln2_w"][:, None] * p["ca_kv_w"][:, :D]
    o["ck_ca"] = p["ln2_b"] @ p["ca_kv_w"][:, :D] + p["ca_kv_b"][:D]
    o["Wv_ca"] = p["ln2_w"][:, None] * p["ca_kv_w"][:, D:]
    cv_ca = p["ln2_b"] @ p["ca_kv_w"][:, D:] + p["ca_kv_b"][D:]
    o["Wo_ca"] = p["ca_o_w"]
    o["W1"] = p["ln3_w"][:, None] * p["ffn1_w"]
    o["ch"] = p["ln3_b"] @ p["ffn1_w"] + p["ffn1_b"]
    o["W2"] = p["ffn2_w"]
    o["c_bias"] = (
        p["sa_o_b"] + p["ca_o_b"] + p["ffn2_b"]
        + cv_sa @ p["sa_o_w"] + cv_ca @ p["ca_o_w"]
    )

    def rope_tabs(sin, cos):
        sinT = np.asarray(sin, F32).T
        cosT = np.asarray(cos, F32).T
        ssg = np.concatenate([-sinT[0:32], sinT[32:64]], 0)
        return np.concatenate([ssg, ssg], 0), np.concatenate([cosT, cosT], 0)

    o["sinT_img"], o["cosT_img"] = rope_tabs(sin_img, cos_img)
    o["sinT_txt"], o["cosT_txt"] = rope_tabs(sin_txt, cos_txt)
    return o


def lhs_pack(W):
    """[D, F] -> [F//128, 128, D//128, 128]: out[ft,p,j,f] = W[j*128+p, ft*128+f]."""
    D, F = W.shape
    return np.ascontiguousarray(
        W.reshape(D // 128, 128, F // 128, 128).transpose(2, 1, 0, 3))


def rhs_pack(W, fc=512):
    """[D, F] -> [F//fc, 128, D//128, fc]: out[c,p,j,f] = W[j*128+p, c*fc+f]."""
    D, F = W.shape
    return np.ascontiguousarray(
        W.reshape(D // 128, 128, F // fc, fc).transpose(2, 1, 0, 3))


# --------------------------------------------------------------------------
# device program
# --------------------------------------------------------------------------
def build(cfg=CFG_FULL):
    import concourse.bass as bass
    import concourse.tile as tile
    from concourse import bacc, mybir
    from concourse.masks import make_identity

    f32 = mybir.dt.float32
    bf16 = mybir.dt.bfloat16
    AF = mybir.ActivationFunctionType
    ALU = mybir.AluOpType

    D, H, FFN, TI, TT, NC = (cfg[k] for k in ("D", "H", "FFN", "TI", "TT", "NC"))
    TPC, TTPC = TI // NC, TT // NC
    JD, JF = D // 128, FFN // 128
    NT = TPC // 128          # img token tiles per core (2)
    HP = H // 2              # head pairs
    E = DH + 1               # v_ext row width per head
    NFC = D // 512           # rhs 512-chunks of D

    nc = bacc.Bacc(None, target_bir_lowering=False, num_devices=NC)

    # ---- I/O ----
    ximg = nc.declare_dram_parameter("ximg", [TPC, D], f32, isOutput=False)
    xtxt = nc.declare_dram_parameter("xtxt", [TTPC, D], f32, isOutput=False)
    cosq = nc.declare_dram_parameter("cosq", [128, TPC], bf16, isOutput=False)
    sinq = nc.declare_dram_parameter("sinq", [128, TPC], bf16, isOutput=False)
    coskt = nc.declare_dram_parameter("coskt", [128, TTPC], bf16, isOutput=False)
    sinkt = nc.declare_dram_parameter("sinkt", [128, TTPC], bf16, isOutput=False)
    W = {}
    for n in ("Wq_sa", "Wk_sa", "Wq_ca", "Wk_ca"):
        W[n] = nc.declare_dram_parameter(n, [JD, 128, JD, 128], bf16, isOutput=False)
    W["W1"] = nc.declare_dram_parameter("W1", [JF, 128, JD, 128], bf16, isOutput=False)
    for n in ("Wv_sa", "Wo_sa", "Wv_ca", "Wo_ca"):
        W[n] = nc.declare_dram_parameter(n, [NFC, 128, JD, 512], bf16, isOutput=False)
    W["W2"] = nc.declare_dram_parameter("W2", [FFN, D], bf16, isOutput=False)
    cq_sa = nc.declare_dram_parameter("cq_sa", [128, JD], f32, isOutput=False)
    ck_sa = nc.declare_dram_parameter("ck_sa", [128, JD], f32, isOutput=False)
    cq_ca = nc.declare_dram_parameter("cq_ca", [128, JD], f32, isOutput=False)
    ck_ca = nc.declare_dram_parameter("ck_ca", [128, JD], f32, isOutput=False)
    ch = nc.declare_dram_parameter("ch", [128, JF], f32, isOutput=False)
    c_bias = nc.declare_dram_parameter("c_bias", [1, D], f32, isOutput=False)
    img_out = nc.declare_dram_parameter("img_out", [TPC, D], f32, isOutput=True)

    # ---- internal DRAM (collective bounce) ----
    HD2 = D // 2
    HE2 = (H // 2) * (DH + 1)
    agk_in = [nc.dram_tensor(f"agk_in{h}", [HD2, TPC], bf16) for h in range(2)]
    agk_out = [nc.dram_tensor(f"agk_out{h}", [NC * HD2, TPC], bf16, addr_space="Shared")
               for h in range(2)]
    agv_in = [nc.dram_tensor(f"agv_in{h}", [TPC, HE2], bf16) for h in range(2)]
    agv_out = [nc.dram_tensor(f"agv_out{h}", [NC * TPC, HE2], bf16, addr_space="Shared")
               for h in range(2)]
    agkt_in = nc.dram_tensor("agkt_in", [D, TTPC], bf16)
    agkt_out = nc.dram_tensor("agkt_out", [NC * D, TTPC], bf16, addr_space="Shared")
    agvt_in = nc.dram_tensor("agvt_in", [TTPC, H * E], bf16)
    agvt_out = nc.dram_tensor("agvt_out", [NC * TTPC, H * E], bf16, addr_space="Shared")
    rg = [list(range(NC))]

    with tile.TileContext(nc) as tc, contextlib.ExitStack() as ctx:
        scope = nc.named_scope
        sing = ctx.enter_context(tc.tile_pool(name="sing", bufs=1))
        big = ctx.enter_context(tc.tile_pool(name="big", bufs=1))
        dbl = ctx.enter_context(tc.tile_pool(name="dbl", bufs=2))
        wp = ctx.enter_context(tc.tile_pool(name="wp", bufs=2))
        work = ctx.enter_context(tc.tile_pool(name="work", bufs=4))
        psA = ctx.enter_context(tc.tile_pool(name="psA", bufs=2, space="PSUM"))
        psPV = ctx.enter_context(tc.tile_pool(name="psPV", bufs=2, space="PSUM"))

        # ---- constants ----
        ident = sing.tile([128, 128], bf16, tag="ident")
        make_identity(nc, ident)
        eps_t = sing.tile([128, 1], f32, tag="eps")
        nc.vector.memset(eps_t, EPS)
        cosq_t = sing.tile([128, TPC], bf16, tag="cosq")
        sinq_t = sing.tile([128, TPC], bf16, tag="sinq")
        nc.sync.dma_start(out=cosq_t, in_=cosq[:, :])
        nc.sync.dma_start(out=sinq_t, in_=sinq[:, :])
        coskt_t = sing.tile([128, TTPC], bf16, tag="coskt")
        sinkt_t = sing.tile([128, TTPC], bf16, tag="sinkt")
        nc.sync.dma_start(out=coskt_t, in_=coskt[:, :])
        nc.sync.dma_start(out=sinkt_t, in_=sinkt[:, :])
        cq_sa_t = sing.tile([128, JD], f32, tag="cq_sa")
        ck_sa_t = sing.tile([128, JD], f32, tag="ck_sa")
        cq_ca_t = sing.tile([128, JD], f32, tag="cq_ca")
        ck_ca_t = sing.tile([128, JD], f32, tag="ck_ca")
        ch_t = sing.tile([128, JF], f32, tag="ch")
        for t, h in ((cq_sa_t, cq_sa), (ck_sa_t, ck_sa), (cq_ca_t, cq_ca), (ck_ca_t, ck_ca), (ch_t, ch)):
            nc.sync.dma_start(out=t, in_=h[:, :])
        cbias_t = sing.tile([128, D], f32, tag="cbias")
        nc.sync.dma_start(out=cbias_t, in_=c_bias[0:1, :].to_broadcast((128, D)))

        # ---- residual stream ----
        ximg_sb = big.tile([128, NT, D], f32, tag="ximg")
        nc.sync.dma_start(out=ximg_sb, in_=ximg[:, :].rearrange("(i p) d -> p i d", p=128))
        xtxt_sb = big.tile([TTPC, D], f32, tag="xtxt")
        nc.sync.dma_start(out=xtxt_sb, in_=xtxt[:, :])

        # ---- helpers ----
        def standardize(x_ap, P, z_out):
            """z_out[:P,:] = (x - mean) * rsqrt(var + eps), bf16.

            rsqrt as exp(-0.5*ln(var+eps)) keeps ScalarE on one table set."""
            nsub = D // 512
            st = work.tile([128, nsub, 6], f32, tag="bnst")
            for s_ in range(nsub):
                nc.vector.bn_stats(out=st[0:P, s_, :], in_=x_ap[:, s_ * 512:(s_ + 1) * 512])
            mv = work.tile([128, 2], f32, tag="bnmv")
            nc.vector.bn_aggr(out=mv[0:P], in_=st[0:P])
            lg = work.tile([128, 1], f32, tag="bnlg")
            nc.scalar.activation(out=lg[0:P], in_=mv[0:P, 1:2], func=AF.Sqrt, bias=eps_t[0:P], scale=1.0)
            a = work.tile([128, 1], f32, tag="bna")
            nc.vector.reciprocal(out=a[0:P], in_=lg[0:P])
            nmua = work.tile([128, 1], f32, tag="bnnm")
            nc.vector.tensor_scalar(out=nmua[0:P], in0=mv[0:P, 0:1], scalar1=a[0:P],
                                    scalar2=-1.0, op0=ALU.mult, op1=ALU.mult)
            nc.vector.tensor_scalar(out=z_out[0:P], in0=x_ap, scalar1=a[0:P],
                                    scalar2=nmua[0:P], op0=ALU.mult, op1=ALU.add)

        def make_zT(x_sb, ntile, P, T, ztag):
            """LN-standardize x_sb [128, ntile, D] then transpose -> [128, JD, T] bf16."""
            zT = dbl.tile([128, JD, T], bf16, tag=ztag)
            for i in range(ntile):
                z = dbl.tile([128, D], bf16, tag="znat")
                x_ap = x_sb[:, i, :] if len(x_sb.shape) == 3 else x_sb[:, :]
                standardize(x_ap, P, z)
                for j in range(JD):
                    pt = psPV.tile([128, 128], bf16, tag="pvtr")
                    nc.tensor.transpose(pt[0:128, 0:P], z[0:P, j * 128:(j + 1) * 128], ident[0:P, 0:P])
                    nc.vector.tensor_copy(out=zT[:, j, i * 128:i * 128 + P], in_=pt[0:128, 0:P])
            return zT

        def rope_apply(out_ap, x_ap, sin_t, cos_t, T):
            rot = work.tile([128, T], bf16, tag="roperot", bufs=2)
            for h0 in (0, 64):
                nc.vector.tensor_copy(out=rot[h0:h0 + 32, :], in_=x_ap[h0 + 32:h0 + 64, :])
                nc.vector.tensor_copy(out=rot[h0 + 32:h0 + 64, :], in_=x_ap[h0:h0 + 32, :])
            nc.vector.tensor_mul(rot, rot, sin_t)
            nc.vector.tensor_mul(out_ap, x_ap, cos_t)
            nc.vector.tensor_add(out_ap, out_ap, rot)

        def proj_T(zT, w_handle, c_t, sin_t, cos_t, T, out, wtag, fts=None):
            """out[:, ft, :T] = rope(W.T @ z + c) for each 128-feature tile."""
            for ft in (range(JD) if fts is None else fts):
                wl = wp.tile([128, JD, 128], bf16, tag=wtag, name=f"wl_{wtag}_{ft}")
                nc.sync.dma_start(out=wl, in_=w_handle[ft])
                ps = psA.tile([128, T], f32, tag="mm", name=f"pp_{wtag}_{ft}")
                for j in range(JD):
                    nc.tensor.matmul(ps, wl[:, j, :], zT[:, j, :], start=(j == 0), stop=(j == JD - 1))
                raw = work.tile([128, T], bf16, tag="praw", bufs=2, name=f"raw_{wtag}_{ft}")
                nc.vector.tensor_scalar(out=raw, in0=ps, scalar1=c_t[:, ft:ft + 1],
                                        scalar2=None, op0=ALU.add)
                rope_apply(out[:, ft, :], raw, sin_t, cos_t, T)

        def vproj_ext(zT, w_handle, ntile, P, vtag, wtag, half_cb=None, fcs=None, tiles=None):
            """v_ext tiles [128, H*E] bf16 (natural layout + ones col per head).

            fc-outer so the first head-half of every token tile completes first;
            half_cb(fc, tiles) fires when a half is fully written (for split AGs)."""
            if tiles is None:
                tiles = []
                for i in range(ntile):
                    ve = dbl.tile([128, H * E], bf16, tag=vtag, name=f"ve_{vtag}_{i}_{wtag}")
                    nc.vector.memset(ve.rearrange("p (h e) -> p h e", e=E)[:, :, DH:E], 1.0)
                    tiles.append(ve)
            for fc in (range(NFC) if fcs is None else fcs):
                wv = wp.tile([128, JD, 512], bf16, tag=wtag, name=f"wv_{wtag}_{fc}_{id(zT)}")
                nc.sync.dma_start(out=wv, in_=w_handle[fc])
                for i in range(ntile):
                    ve = tiles[i]
                    ps = psA.tile([128, 512], f32, tag="mm", name=f"vp_{vtag}_{i}_{fc}")
                    for j in range(JD):
                        nc.tensor.matmul(ps[0:P, :], zT[:, j, i * 128:i * 128 + P], wv[:, j, :],
                                         start=(j == 0), stop=(j == JD - 1))
                    for hh in range(512 // DH):
                        h = fc * (512 // DH) + hh
                        nc.vector.tensor_copy(out=ve[0:P, h * E:h * E + DH],
                                              in_=ps[0:P, hh * DH:(hh + 1) * DH])
                if half_cb is not None:
                    half_cb(fc, tiles)
            return tiles

        # ================== SA K/V (own tokens) + split AllGathers ==================
        with scope("ln1"):
            z1T = make_zT(ximg_sb, NT, 128, TPC, "zT1")
        kT_own = dbl.tile([128, JD, TPC], bf16, tag="kTo")
        v_tiles = []
        for i in range(NT):
            ve = dbl.tile([128, H * E], bf16, tag="vext", name=f"ve_sa_{i}")
            nc.vector.memset(ve.rearrange("p (h e) -> p h e", e=E)[:, :, DH:E], 1.0)
            v_tiles.append(ve)

        def k_half(hh):
            nc.sync.dma_start(
                out=agk_in[hh][:, :].rearrange("(j p) t -> p j t", p=128),
                in_=kT_own[:, hh * (JD // 2):(hh + 1) * (JD // 2), :])
            nc.gpsimd.collective_compute("AllGather", ALU.bypass, replica_groups=rg,
                                         ins=[agk_in[hh][:, :].opt()],
                                         outs=[agk_out[hh][:, :].opt()])

        def v_half_done(fc, tiles):
            for i, ve in enumerate(tiles):
                nc.sync.dma_start(out=agv_in[fc][i * 128:(i + 1) * 128, :],
                                  in_=ve[:, fc * HE2:(fc + 1) * HE2])
            nc.gpsimd.collective_compute("AllGather", ALU.bypass, replica_groups=rg,
                                         ins=[agv_in[fc][:, :].opt()],
                                         outs=[agv_out[fc][:, :].opt()])

        with scope("kv_sa"):
            proj_T(z1T, W["Wk_sa"], ck_sa_t, sinq_t, cosq_t, TPC, kT_own, "wk", fts=range(0, JD // 2))
        k_half(0)
        with scope("v_sa"):
            vproj_ext(z1T, W["Wv_sa"], NT, 128, "vext", "wv", half_cb=v_half_done,
                      fcs=[0], tiles=v_tiles)
        with scope("kv_sa"):
            proj_T(z1T, W["Wk_sa"], ck_sa_t, sinq_t, cosq_t, TPC, kT_own, "wk", fts=range(JD // 2, JD))
        k_half(1)
        with scope("v_sa"):
            vproj_ext(z1T, W["Wv_sa"], NT, 128, "vext", "wv", half_cb=v_half_done,
                      fcs=[1], tiles=v_tiles)
        # ================== SA q (overlaps AGs) ==================
        qT = dbl.tile([128, JD, TPC], bf16, tag="qT")
        with scope("q_sa"):
            proj_T(z1T, W["Wq_sa"], cq_sa_t, sinq_t, cosq_t, TPC, qT, "wk")

        # ================== CA txt K/V (fills the AG wait window) ==================
        with scope("txtkv"):
            z2tT = make_zT(xtxt_sb, 1, TTPC, TTPC, "zTt")
            ktT_own = dbl.tile([128, JD, TTPC], bf16, tag="kTo")
            proj_T(z2tT, W["Wk_ca"], ck_ca_t, sinkt_t, coskt_t, TTPC, ktT_own, "wk")
        nc.sync.dma_start(out=agkt_in[:, :].rearrange("(j p) t -> p j t", p=128), in_=ktT_own)
        nc.gpsimd.collective_compute("AllGather", ALU.bypass, replica_groups=rg,
                                     ins=[agkt_in[:, :].opt()], outs=[agkt_out[:, :].opt()])
        with scope("txtkv"):
            vt_tiles = vproj_ext(z2tT, W["Wv_ca"], 1, TTPC, "vext", "wv")
        nc.sync.dma_start(out=agvt_in[:, :], in_=vt_tiles[0][0:TTPC, :])
        nc.gpsimd.collective_compute("AllGather", ALU.bypass, replica_groups=rg,
                                     ins=[agvt_in[:, :].opt()], outs=[agvt_out[:, :].opt()])

        # ================== gathered K/V (per half) ==================
        KH = []
        for hh in range(2):
            ka = big.tile([128, JD // 2, NC, TPC], bf16, tag=f"KALL{hh}", name=f"KALL{hh}")
            for cc in range(NC):
                nc.sync.dma_start(out=ka[:, :, cc, :],
                                  in_=agk_out[hh][cc * HD2:(cc + 1) * HD2, :]
                                  .rearrange("(j p) t -> p j t", p=128))
            KH.append(ka)
        VH = []
        for hh in range(2):
            va = big.tile([128, NC * NT, HE2], bf16, tag=f"VALL{hh}", name=f"VALL{hh}")
            nc.sync.dma_start(out=va, in_=agv_out[hh][:, :].rearrange("(u p) e -> p u e", p=128))
            VH.append(va)
        KALL, VALL = KH, VH

        def attention(qT_t, KA, VA, n_tok_blk, attnT):
            split = isinstance(KA, list)
            for g in range(HP):
                h0, h1 = 2 * g, 2 * g + 1
                if split:
                    KAg = KA[g // (HP // 2)]
                    VAg = VA[g // (HP // 2)]
                    gg, hh0, hh1 = g % (HP // 2), h0 % (H // 2), h1 % (H // 2)
                else:
                    KAg, VAg, gg, hh0, hh1 = KA, VA, g, h0, h1
                pv0 = psPV.tile([DH + 1, TPC], f32, tag="pvtr", name=f"pv0_{g}")
                pv1 = psPV.tile([DH + 1, TPC], f32, tag="pvtr", name=f"pv1_{g}")
                if n_tok_blk >= 128:
                    sub = n_tok_blk // 128
                    ntile = NC * sub
                    pTs = []
                    for b in range(ntile):
                        cc, jj = b // sub, b % sub
                        ps = psA.tile([128, 2, 512], f32, tag="sc2", name=f"sc_{g}_{b}")
                        ksl = KAg[:, gg, cc, jj * 128:(jj + 1) * 128]
                        nc.tensor.matmul(ps[:, 0, 0:TPC], ksl[0:64, :], qT_t[0:64, g, :], start=True, stop=True)
                        nc.tensor.matmul(ps[:, 1, 0:TPC], ksl[64:128, :], qT_t[64:128, g, :], start=True, stop=True)
                        pT = work.tile([128, 2 * TPC], bf16, tag="pT", bufs=NC * NT + 1,
                                       name=f"pt_{g}_{b}")
                        nc.scalar.activation(out=pT, in_=ps[:, :, 0:TPC], func=AF.Exp)
                        pTs.append(pT)
                    for b in range(ntile):
                        cc, jj = b // sub, b % sub
                        u = (cc * n_tok_blk) // 128 + jj
                        nc.tensor.matmul(pv0, VAg[:, u, hh0 * E:(hh0 + 1) * E], pTs[b][:, 0:TPC],
                                         start=(b == 0), stop=(b == ntile - 1))
                        nc.tensor.matmul(pv1, VAg[:, u, hh1 * E:(hh1 + 1) * E], pTs[b][:, TPC:2 * TPC],
                                         start=(b == 0), stop=(b == ntile - 1))
                else:
                    assert n_tok_blk == 64
                    pTs = []
                    for b2 in range(NC // 2):
                        ps = psA.tile([128, 2, 512], f32, tag="sc2", name=f"scc_{g}_{b2}")
                        for s_ in range(2):
                            cc = 2 * b2 + s_
                            ksl = KAg[:, gg, cc, :]
                            nc.tensor.matmul(ps[s_ * 64:(s_ + 1) * 64, 0, 0:TPC], ksl[0:64, :],
                                             qT_t[0:64, g, :], start=True, stop=True,
                                             tile_position=(0, s_ * 64))
                            nc.tensor.matmul(ps[s_ * 64:(s_ + 1) * 64, 1, 0:TPC], ksl[64:128, :],
                                             qT_t[64:128, g, :], start=True, stop=True,
                                             tile_position=(64, s_ * 64))
                        pT = work.tile([128, 2 * TPC], bf16, tag="pT", bufs=NC * NT + 1,
                                       name=f"ptc_{g}_{b2}")
                        nc.scalar.activation(out=pT, in_=ps[:, :, 0:TPC], func=AF.Exp)
                        pTs.append(pT)
                    for b2 in range(NC // 2):
                        nc.tensor.matmul(pv0, VAg[:, b2, hh0 * E:(hh0 + 1) * E], pTs[b2][:, 0:TPC],
                                         start=(b2 == 0), stop=(b2 == NC // 2 - 1))
                        nc.tensor.matmul(pv1, VAg[:, b2, hh1 * E:(hh1 + 1) * E], pTs[b2][:, TPC:2 * TPC],
                                         start=(b2 == 0), stop=(b2 == NC // 2 - 1))
                for h, pv in ((h0, pv0), (h1, pv1)):
                    rec = work.tile([1, TPC], f32, tag="rec", name=f"rec_{g}_{h}")
                    nc.vector.reciprocal(out=rec, in_=pv[DH:DH + 1, :])
                    recb = work.tile([64, TPC], f32, tag="recb", bufs=2, name=f"recb_{g}_{h}")
                    nc.gpsimd.partition_broadcast(recb, rec[0:1, :])
                    nc.vector.tensor_mul(attnT[(h % 2) * 64:(h % 2) * 64 + 64, h // 2, :],
                                         pv[0:DH, :], recb)

        def oproj_residual(attnT, w_handle):
            for fc in range(NFC):
                wo = wp.tile([128, JD, 512], bf16, tag="wv", name=f"wo_{fc}")
                nc.sync.dma_start(out=wo, in_=w_handle[fc])
                for i in range(NT):
                    ps = psA.tile([128, 512], f32, tag="mm", name=f"op_{fc}_{i}")
                    for j in range(JD):
                        nc.tensor.matmul(ps, attnT[:, j, i * 128:(i + 1) * 128], wo[:, j, :],
                                         start=(j == 0), stop=(j == JD - 1))
                    nc.vector.tensor_add(ximg_sb[:, i, fc * 512:(fc + 1) * 512],
                                         ximg_sb[:, i, fc * 512:(fc + 1) * 512], ps)

        # ================== SA attention + o-proj ==================
        attnT = dbl.tile([128, JD, TPC], bf16, tag="attnT")
        with scope("sa_attn"):
            attention(qT, KALL, VALL, TPC, attnT)
        with scope("sa_oproj"):
            oproj_residual(attnT, W["Wo_sa"])

        # ================== CA ==================
        with scope("ln2"):
            z2T = make_zT(ximg_sb, NT, 128, TPC, "zT1")
        qTc = dbl.tile([128, JD, TPC], bf16, tag="qT")
        with scope("q_ca"):
            proj_T(z2T, W["Wq_ca"], cq_ca_t, sinq_t, cosq_t, TPC, qTc, "wk")
        KTALL = big.tile([128, JD, NC, TTPC], bf16, tag="KTALL")
        for cc in range(NC):
            nc.sync.dma_start(out=KTALL[:, :, cc, :], in_=agkt_out[cc * D:(cc + 1) * D, :]
                              .rearrange("(j p) t -> p j t", p=128))
        VTALL = big.tile([128, (NC * TTPC) // 128, H * E], bf16, tag="VTALL")
        nc.sync.dma_start(out=VTALL, in_=agvt_out[:, :].rearrange("(u p) e -> p u e", p=128))
        attnTc = dbl.tile([128, JD, TPC], bf16, tag="attnT")
        with scope("ca_attn"):
            attention(qTc, KTALL, VTALL, TTPC, attnTc)
        with scope("ca_oproj"):
            oproj_residual(attnTc, W["Wo_ca"])

        # ================== FFN ==================
        with scope("ln3"):
            z3T = make_zT(ximg_sb, NT, 128, TPC, "zT1")
        hT = big.tile([128, JF, TPC], bf16, tag="hT")
        with scope("ffn1"):
            for ft in range(JF):
                wl = wp.tile([128, JD, 128], bf16, tag="wk", name=f"w1_{ft}")
                nc.sync.dma_start(out=wl, in_=W["W1"][ft])
                ps = psA.tile([128, TPC], f32, tag="mm", name=f"f1_{ft}")
                for j in range(JD):
                    nc.tensor.matmul(ps, wl[:, j, :], z3T[:, j, :], start=(j == 0), stop=(j == JD - 1))
                nc.scalar.activation(out=hT[:, ft, :], in_=ps, func=AF.Silu,
                                     bias=ch_t[:, ft:ft + 1], scale=1.0)
        with scope("ffn2"):
            acct = [psA.tile([128, NFC, 512], f32, tag="sc2", name=f"acct_{i}")
                    for i in range(NT)]
            accs = [[acct[i][:, fc, :] for fc in range(NFC)] for i in range(NT)]
            for j in range(JF):
                w2t = wp.tile([128, D], bf16, tag="w2t", name=f"w2_{j}")
                nc.sync.dma_start(out=w2t, in_=W["W2"][j * 128:(j + 1) * 128, :])
                for i in range(NT):
                    for fc in range(NFC):
                        nc.tensor.matmul(accs[i][fc], hT[:, j, i * 128:(i + 1) * 128],
                                         w2t[:, fc * 512:(fc + 1) * 512],
                                         start=(j == 0), stop=(j == JF - 1))
            for i in range(NT):
                for fc in range(NFC):
                    nc.vector.tensor_add(ximg_sb[:, i, fc * 512:(fc + 1) * 512],
                                         ximg_sb[:, i, fc * 512:(fc + 1) * 512], accs[i][fc])
                nc.vector.tensor_add(ximg_sb[:, i, :], ximg_sb[:, i, :], cbias_t)
                nc.sync.dma_start(out=img_out[i * 128:(i + 1) * 128, :], in_=ximg_sb[:, i, :])

    nc.compile()
    return nc


# --------------------------------------------------------------------------
# per-core input maps
# --------------------------------------------------------------------------
def make_in_maps(img_seq, text_seq, w, cfg=CFG_FULL):
    D, H, FFN, TI, TT, NC = (cfg[k] for k in ("D", "H", "FFN", "TI", "TT", "NC"))
    TPC, TTPC = TI // NC, TT // NC
    img = np.asarray(img_seq, F32).reshape(TI, D)
    txt = np.asarray(text_seq, F32).reshape(TT, D)

    def pack_c(v):
        return np.ascontiguousarray(v.reshape(-1, 128).T.astype(F32))

    shared = {
        "Wq_sa": lhs_pack(w["Wq_sa"].astype(BF16)), "Wk_sa": lhs_pack(w["Wk_sa"].astype(BF16)),
        "Wq_ca": lhs_pack(w["Wq_ca"].astype(BF16)), "Wk_ca": lhs_pack(w["Wk_ca"].astype(BF16)),
        "W1": lhs_pack(w["W1"].astype(BF16)),
        "Wv_sa": rhs_pack(w["Wv_sa"].astype(BF16)), "Wo_sa": rhs_pack(w["Wo_sa"].astype(BF16)),
        "Wv_ca": rhs_pack(w["Wv_ca"].astype(BF16)), "Wo_ca": rhs_pack(w["Wo_ca"].astype(BF16)),
        "W2": np.ascontiguousarray(w["W2"].astype(BF16)),
        "cq_sa": pack_c(w["cq_sa"]), "ck_sa": pack_c(w["ck_sa"]),
        "cq_ca": pack_c(w["cq_ca"]), "ck_ca": pack_c(w["ck_ca"]),
        "ch": pack_c(w["ch"]), "c_bias": w["c_bias"].reshape(1, D).astype(F32),
    }
    shared = {k: np.ascontiguousarray(v) for k, v in shared.items()}
    maps = []
    for c in range(NC):
        m = dict(shared)
        m["ximg"] = np.ascontiguousarray(img[c * TPC:(c + 1) * TPC])
        m["xtxt"] = np.ascontiguousarray(txt[c * TTPC:(c + 1) * TTPC])
        m["cosq"] = np.ascontiguousarray(w["cosT_img"][:, c * TPC:(c + 1) * TPC].astype(BF16))
        m["sinq"] = np.ascontiguousarray(w["sinT_img"][:, c * TPC:(c + 1) * TPC].astype(BF16))
        m["coskt"] = np.ascontiguousarray(w["cosT_txt"][:, c * TTPC:(c + 1) * TTPC].astype(BF16))
        m["sinkt"] = np.ascontiguousarray(w["sinT_txt"][:, c * TTPC:(c + 1) * TTPC].astype(BF16))
        maps.append(m)
    return maps


_CACHE = {}
last_exec_ns = None
last_result = None


def kernel(img_seq, text_seq, sin_img, cos_img, sin_txt, cos_txt, params):
    global last_exec_ns, last_result
    cfg = CFG_FULL
    trace = os.environ.get("KERNEL_TRACE", "0") == "1"
    if trace:
        _install_ntff_shim()
    from concourse.bass_utils import run_bass_kernel_spmd

    if "nc" not in _CACHE:
        _CACHE["nc"] = build(cfg)
    nc = _CACHE["nc"]
    w = host_prep(params, sin_img, cos_img, sin_txt, cos_txt, cfg)
    in_maps = make_in_maps(img_seq, text_seq, w, cfg)
    res = run_bass_kernel_spmd(nc, in_maps, core_ids=list(range(cfg["NC"])), trace=trace)
    last_result = res
    last_exec_ns = res.exec_time_ns
    TI, D, NC = cfg["TI"], cfg["D"], cfg["NC"]
    TPC = TI // NC
    img = np.concatenate([res.results[c]["img_out"] for c in range(NC)], 0)
    img = img.reshape(1, TI, D).astype(F32)
    txt = np.asarray(text_seq, F32)
    return img, txt
